# revision 11
# baseline (speedup 1.0000x reference)
"""DoRA linear layer (nn_DoraLinearLayer) on 8 Trainium2 NeuronCores.

Math: out = (s-1)*(x @ W.T) + 2*s*((x @ A.T) @ B.T),
      s = magnitude / ||W + 2*B@A||_row  (stop-grad norm)

This factors exactly into ONE matmul per token: out = x @ Weff.T with
      n2     = ||W||²_row + Σ_r (2B).T ∘ (2A@W.T + G@(2B).T)   (G = A@A.T)
      s      = magnitude / sqrt(n2)
      Weff.T = (s-1) ∘ W.T + A.T @ (s∘(2B).T)
Device tensors carry a host-side ×16 scale (wt16, b2t16, mag16) so the
squares land in fp8e4m3 range; the 1/16 folds into the PSUM drain.
G is a 16×16 host-marshaled Gram of the rank-16 adapter.

Main loop is hybrid-precision: the leading 8 of 32 contraction chunks
are fp16 MMs; the trailing 24 run as 12 fp8e4m3 DoubleRow pair-MMs
(K=256 per instruction, ~1.13x the per-MM cost for 2x the contraction)
against weff tiles built directly in fp8. Validated numerically:
rel err ~1.8e-2 vs the 2e-2 gate (model matches HW to ~2%).

Queue discipline (each engine's DMA blocks its queue for the duration
of the in-flight transfer, so duties must not collide):
  sync:   wt chunks 0-15 (fat 4-chunk waves), then ALL fp16 x slabs
  gpsimd: wt chunks 16-31, then square/feeder-mul shares
  scalar: adapter smalls + ACT-table warmups, square share, s-chain
          sqrt/sm1, fp8 x slabs (split in half), drains + out DMAs
  vector: warmup memsets, square share, s-chain, weff feeder
The norm pipeline consumes chunks in the interleaved landing order
(sync and gpsimd waves alternate), a widened warmup burst keeps the
HAM clock at full rate through the DMA-paced phase, and filler
matmuls bridge the s-chain latency. The weff build trails 3 chunks
ahead of the chunk-major first token group; trailing-chunk weff goes
straight to fp8 (DVE tensor_add with fp8 output — no fp16 write, no
separate cast).

Sharding: column-parallel over out_features — core i owns rows
[i*512, (i+1)*512) of W/B/magnitude, x and A replicated, output shard
concatenated on the last dim on the host. Host-side work is marshaling
only: casts to fp16/fp8, transposes, slicing, static scaling, the
16×16 adapter Gram.
"""
import numpy as np

import concourse.bass as bass
import concourse.tile as tile
from concourse import bacc, mybir
from concourse.bass_utils import run_bass_kernel_spmd

N_CORES = 8
TOKENS, D_IN, D_OUT, R = 8192, 4096, 4096, 16
O = D_OUT // N_CORES          # 512 output features per core
P = 128                       # partitions
NCH = D_IN // P               # 32 contraction chunks
N_F8 = 24                     # trailing chunks computed in fp8 DoubleRow
NCH16 = NCH - N_F8            # leading fp16 chunks
NP8 = N_F8 // 2               # DoubleRow pair-MMs per tile
SCALING = 2.0                 # lora_alpha / r
SC16 = 16.0                   # static ×16 device scale (fp8 sq range)
N_WARM = 12                   # PE warmup matmuls (HAM ramp during DMA)

# token groups: first is chunk-major (7 PSUM banks) so the matmuls
# trail the weff feeder; the rest are tile-major; small last group
TGROUPS = [(0, 768, True)]
_t = 768
while _t + 512 <= TOKENS - 256:
    TGROUPS.append((_t, 512, False))
    _t += 512
TGROUPS.append((_t, 256, False))

f16 = mybir.dt.float16
f32 = mybir.dt.float32
f8 = mybir.dt.float8e4
Square = mybir.ActivationFunctionType.Square
Copy = mybir.ActivationFunctionType.Copy

_CACHE: dict = {}


def emit_kernel(nc, tc, xg16, xg8, wt, a, a2t, g, b2t, mag, out):
    """Emit the per-core program. All DRAM APs are per-core shapes."""
    from contextlib import ExitStack

    DoubleRow = mybir.MatmulPerfMode.DoubleRow

    with ExitStack() as ctx:
        singles = ctx.enter_context(tc.tile_pool(name="singles", bufs=1))
        setup = ctx.enter_context(tc.tile_pool(name="setup", bufs=8))
        # 8 PSUM banks: gen(6: warm + main mm) + lws(1) + nh(1: n2 rows
        # 0:16, h rows 32:48, then the full-width s broadcast)
        ps_gen = ctx.enter_context(tc.tile_pool(name="ps_gen", bufs=5, space="PSUM"))
        ps_lws = ctx.enter_context(tc.tile_pool(name="ps_lws", bufs=2, space="PSUM"))
        ps_nh = ctx.enter_context(tc.tile_pool(name="ps_nh", bufs=1, space="PSUM"))
        xpool = ctx.enter_context(tc.tile_pool(name="xpool", bufs=2))
        x0pool = ctx.enter_context(tc.tile_pool(name="x0pool", bufs=1))
        x8pool = ctx.enter_context(tc.tile_pool(name="x8pool", bufs=2))
        opool = ctx.enter_context(tc.tile_pool(name="opool", bufs=3))

        # ---- adapter smalls + ACT-table warmups on scalar: a2t first
        # (the h-chain needs it as soon as wt chunk 0 lands), then the
        # two table loads, then the rest of the smalls.
        a2t_sb = singles.tile([P, NCH, R], f16)
        nc.scalar.dma_start(out=a2t_sb, in_=a2t.rearrange("p (c r) -> p c r", r=R))
        act_warm = singles.tile([1, 1], f32)
        nc.vector.memset(act_warm, 1.0)
        act_warm2 = singles.tile([1, 1], f32)
        nc.scalar.sqrt(act_warm2, act_warm)
        act_warm3 = singles.tile([1, 1], f32)
        nc.scalar.activation(act_warm3, act_warm, Square)
        a_sb = singles.tile([R, D_IN], f16)
        nc.scalar.dma_start(out=a_sb, in_=a)
        b2t_sb = singles.tile([R, O], f16)
        nc.scalar.dma_start(out=b2t_sb, in_=b2t)
        g_sb = singles.tile([R, R], f16)
        nc.scalar.dma_start(out=g_sb, in_=g)
        mag_sb = singles.tile([1, O], f32)
        nc.scalar.dma_start(out=mag_sb, in_=mag)

        # ---- 16·W.T as fat 4-chunk waves: sync takes chunks 0-15,
        # gpsimd takes 16-31 (both queues stream concurrently; each
        # queue's later DMAs ring-block behind these, which is exactly
        # the priority we want).
        wt_sb = singles.tile([P, NCH, O], f16)
        wt_r = wt.rearrange("p (c o) -> p c o", o=O)
        wave_sizes = [2, 2, 4, 4, 4]
        lo = 0
        for wsz in wave_sizes:
            nc.sync.dma_start(out=wt_sb[:, lo:lo + wsz, :],
                              in_=wt_r[:, lo:lo + wsz, :])
            lo += wsz
        lo = 16
        for wsz in wave_sizes:
            nc.gpsimd.dma_start(out=wt_sb[:, lo:lo + wsz, :],
                                in_=wt_r[:, lo:lo + wsz, :])
            lo += wsz
        wt_t = [wt_sb[:, c, :] for c in range(NCH)]

        # ---- warmup operand memsets (vector queue) ----
        ones128 = singles.tile([P, P], f16)
        nc.vector.memset(ones128, 1.0)
        warm_rhs = singles.tile([P, O], f16)
        nc.vector.memset(warm_rhs, 0.002)
        ones8 = singles.tile([P, 2, 16], f8)
        nc.vector.memset(ones8, 1.0)
        ones16 = singles.tile([R, R], f16)
        nc.vector.memset(ones16, 1.0)
        ones_row32 = singles.tile([1, P], f32)
        nc.vector.memset(ones_row32, 1.0)

        # ---- PE warmup: full-K fp16 matmuls start the clock ramp and
        # keep HAM busy while the wt waves land
        warm_ps = ps_gen.tile([P, O], f32, name="gen")
        for _ in range(N_WARM):
            nc.tensor.matmul(warm_ps, lhsT=ones128, rhs=warm_rhs,
                             start=True, stop=True)

        # ---- norm pipeline, trailing the wt DMA in interleaved chunk
        # landing order (sync delivers 0..15, gpsimd 16..31):
        #   V/S/G: sq[c] = wt16[c]²  (fp8; shares 12/13/7 balance the
        #   engines' speeds and queue-free times)
        #   PE: h += (2A).T_c-major @ wt16[c]   (rows 32:48 of nh bank)
        #   PE: n2 += ones @ sq-pair  (fp8 DoubleRow, rows 0:16)
        sq_sb = singles.tile([P, NCH, O], f8)
        nh = ps_nh.tile([P, O], f32, name="nh")
        n2_ap = nh[0:16, :]
        h_ap = nh[32:48, :]

        c_seq = []
        for i in range(16):
            c_seq.append(i)
            c_seq.append(16 + i)
        # engine shares: vector 12, scalar 13, gpsimd 7 (by position)
        sq_eng = {}
        v_n = s_n = g_n = 0
        for p_, c in enumerate(c_seq):
            if g_n < 7 and p_ % 4 == 1:
                sq_eng[c] = "g"; g_n += 1
            elif s_n < 13 and p_ % 2 == 1:
                sq_eng[c] = "s"; s_n += 1
            elif v_n < 12:
                sq_eng[c] = "v"; v_n += 1
            else:
                sq_eng[c] = "s"; s_n += 1

        def emit_n2(k):
            nc.tensor.matmul(n2_ap, lhsT=ones8,
                             rhs=sq_sb[:, 2 * k:2 * k + 2, :],
                             perf_mode=DoubleRow,
                             start=(k == 0), stop=False)

        def fill_one():
            nc.tensor.matmul(warm_ps, lhsT=ones128, rhs=warm_rhs,
                             start=True, stop=True)

        pair_order = []
        for i in range(8):
            pair_order.append(i)
            pair_order.append(8 + i)
        sq_pos = {}
        n2_ptr = 0
        for p_, c in enumerate(c_seq):
            e = sq_eng[c]
            if e == "v":
                nc.vector.tensor_mul(sq_sb[:, c, :], wt_t[c], wt_t[c])
            elif e == "s":
                nc.scalar.activation(sq_sb[:, c, :], wt_t[c], Square)
            else:
                nc.gpsimd.tensor_mul(sq_sb[:, c, :], wt_t[c], wt_t[c])
            sq_pos[c] = p_
            nc.tensor.matmul(h_ap, lhsT=a2t_sb[:, c, :], rhs=wt_t[c],
                             start=(p_ == 0), stop=False)
            # emit n2 pairs as their squares age past a small lag
            if n2_ptr < len(pair_order):
                k = pair_order[n2_ptr]
                ca, cb = 2 * k, 2 * k + 1
                if (ca in sq_pos and cb in sq_pos
                        and p_ >= max(sq_pos[ca], sq_pos[cb]) + 6):
                    emit_n2(k)
                    n2_ptr += 1
            if p_ in (6, 10, 14, 18, 22, 24):
                fill_one()
        for k in pair_order[n2_ptr:]:
            emit_n2(k)

        # G-term folded onto the h chain, then the correction row-sum.
        # Filler matmuls between the dependency hops keep the PE dense
        # through the chain latency (an idle PE drops to half clock).
        def fill(n):
            for _ in range(n):
                nc.tensor.matmul(warm_ps, lhsT=ones128, rhs=warm_rhs,
                                 start=True, stop=True)

        nc.tensor.matmul(h_ap, lhsT=g_sb, rhs=b2t_sb, start=False, stop=True)
        fill(3)
        hterm = singles.tile([R, O], f16)
        nc.vector.tensor_mul(hterm, b2t_sb, h_ap)
        nc.tensor.matmul(n2_ap, lhsT=ones16, rhs=hterm, start=False, stop=True)
        fill(14)

        # ---- s = mag16 / sqrt(n2); broadcast into the nh bank ----
        nrm = singles.tile([1, O], f32)
        nc.scalar.sqrt(nrm, nh[0:1, :])
        rn = singles.tile([1, O], f32)
        nc.vector.reciprocal_approx_fast(out=rn, in_=nrm)
        s_row = singles.tile([1, O], f32)
        nc.vector.tensor_mul(s_row, mag_sb, rn)
        nc.tensor.matmul(nh, lhsT=ones_row32, rhs=s_row, start=True, stop=True)
        fill(6)
        b2st = singles.tile([R, O], f16)
        nc.vector.tensor_mul(b2st, b2t_sb, nh[0:R, :])
        sm1_sb = singles.tile([P, O], f32)
        nc.scalar.activation(sm1_sb, nh, Copy, bias=-1.0)

        # ---- x prefetch: all fp16 slabs on sync (ring-blocked behind
        # sync's wt waves = the priority we want); fp8 slabs on scalar,
        # split in half so a slab never blocks the drain stream long.
        off16 = []
        off8 = []
        o16 = o8 = 0
        for (t0_, nt_, _) in TGROUPS:
            off16.append(o16)
            off8.append(o8)
            o16 += NCH16 * nt_
            o8 += N_F8 * nt_

        def x16_ap(gi):
            nt_ = TGROUPS[gi][1]
            return xg16[:, off16[gi]: off16[gi] + NCH16 * nt_].rearrange(
                "p (c t) -> p c t", t=nt_)

        def x8_ap(gi):
            nt_ = TGROUPS[gi][1]
            return xg8[:, off8[gi]: off8[gi] + N_F8 * nt_].rearrange(
                "p (k j t) -> p k j t", j=2, t=nt_)

        def x8_fetch(gi, tile_):
            ap = x8_ap(gi)
            h1 = NP8 // 2
            nc.sync.dma_start(out=tile_[:, :h1], in_=ap[:, :h1])
            nc.sync.dma_start(out=tile_[:, h1:], in_=ap[:, h1:])

        ntok0 = TGROUPS[0][1]
        xt0 = x0pool.tile([P, NCH16, ntok0], f16, name="xt0")
        ap0 = x16_ap(0)
        for w in range(NCH16 // 4):
            lo, hi = 4 * w, 4 * w + 4
            nc.sync.dma_start(out=xt0[:, lo:hi, :], in_=ap0[:, lo:hi, :])
        x80 = x8pool.tile([P, NP8, 2, ntok0], f8, name="x8")
        x8_fetch(0, x80)
        ntok1 = TGROUPS[1][1]
        xt1 = xpool.tile([P, NCH16, ntok1], f16, name="xt")
        nc.sync.dma_start(out=xt1, in_=x16_ap(1))
        x81 = x8pool.tile([P, NP8, 2, ntok1], f8, name="x8")
        x8_fetch(1, x81)
        xt_pre = {0: (xt0, x80), 1: (xt1, x81)}

        # ---- weff build feeder: lws = A.T_c @ (s∘b2t16) on a rotating
        # bank. Leading (fp16) chunks: weff16[c] = sm1∘wt16[c] + lws in
        # place over wt16 (both DVE ops). Trailing chunks build straight
        # into fp8: tmp = sm1∘wt16[c] (vector/gpsimd alternating), then
        # weff8[c] = tmp + lws with fp8 output on the DVE.
        weff8 = singles.tile([P, N_F8, O], f8)

        def emit_weff(c):
            lws = ps_lws.tile([P, O], f32, name="lws")
            nc.tensor.matmul(lws, lhsT=a_sb[:, c * P:(c + 1) * P], rhs=b2st,
                             start=True, stop=True)
            tmp = setup.tile([P, O], f32, name="tmp")
            if c == 3 or c % 4 == 1:
                nc.vector.tensor_mul(tmp, wt_t[c], sm1_sb)
            else:
                nc.gpsimd.tensor_mul(tmp, wt_t[c], sm1_sb)
            if c < NCH16:
                nc.vector.tensor_add(wt_t[c], tmp, lws)
            else:
                nc.vector.tensor_add(weff8[:, c - NCH16, :], tmp, lws)

        weff_t = wt_t
        emit_weff(0)
        fill(4)
        emit_weff(1)
        fill(4)
        emit_weff(2)
        fill(5)

        def emit_pair_mm(ps, x8_t, m, k, stop):
            nc.tensor.matmul(
                ps,
                lhsT=x8_t[:, k, :, m * P: (m + 1) * P],
                rhs=weff8[:, 2 * k:2 * k + 2, :],
                perf_mode=DoubleRow,
                start=False, stop=stop,
            )

        for gi, (t0, ntok, chunk_major) in enumerate(TGROUPS):
            nm = ntok // P
            if gi in xt_pre:
                xt_t, x8_t = xt_pre[gi]
            else:
                xt_t = xpool.tile([P, NCH16, ntok], f16, name="xt")
                nc.sync.dma_start(out=xt_t, in_=x16_ap(gi))
                x8_t = x8pool.tile([P, NP8, 2, ntok], f8, name="x8")
                x8_fetch(gi, x8_t)
            if chunk_major:
                # the 6th accumulator reuses the norm bank (dead by now)
                pss = [ps_gen.tile([P, O], f32, name="gen") for _ in range(nm - 1)]
                pss.append(ps_nh.tile([P, O], f32, name="nh"))
                for c in range(NCH16):
                    if c + 3 < NCH:
                        emit_weff(c + 3)
                    for m in range(nm):
                        nc.tensor.matmul(
                            pss[m],
                            lhsT=xt_t[:, c, m * P: (m + 1) * P],
                            rhs=weff_t[c],
                            start=(c == 0), stop=False,
                        )
                # remaining weff builds interleave with the pair phase
                pend = list(range(NCH16 + 3, NCH))
                for fc in pend[:3]:
                    emit_weff(fc)
                pend = pend[3:]
                for k in range(NP8):
                    for fc in pend[:2]:
                        emit_weff(fc)
                    pend = pend[2:]
                    for m in range(nm):
                        emit_pair_mm(pss[m], x8_t, m, k, k == NP8 - 1)
                for m in range(nm):
                    ot = opool.tile([P, O], f32, name="ot")
                    nc.scalar.activation(ot, pss[m], Copy, scale=1.0 / SC16)
                    nc.scalar.dma_start(
                        out=out[t0 + m * P: t0 + (m + 1) * P, :], in_=ot)
            else:
                for m in range(nm):
                    ps = ps_gen.tile([P, O], f32, name="gen")
                    for c in range(NCH16):
                        nc.tensor.matmul(
                            ps,
                            lhsT=xt_t[:, c, m * P: (m + 1) * P],
                            rhs=weff_t[c],
                            start=(c == 0), stop=False,
                        )
                    for k in range(NP8):
                        emit_pair_mm(ps, x8_t, m, k, k == NP8 - 1)
                    ot = opool.tile([P, O], f32, name="ot")
                    last = (gi == len(TGROUPS) - 1 and m == nm - 1)
                    if last:
                        # final tile drains on the idle vector engine and
                        # sync queue so the tail isn't serialized behind
                        # the previous tile's scalar-queue work
                        nc.vector.tensor_scalar_mul(ot, ps, 1.0 / SC16)
                        nc.sync.dma_start(
                            out=out[t0 + m * P: t0 + (m + 1) * P, :], in_=ot)
                    else:
                        nc.scalar.activation(ot, ps, Copy, scale=1.0 / SC16)
                        nc.scalar.dma_start(
                            out=out[t0 + m * P: t0 + (m + 1) * P, :], in_=ot)


def build_nc():
    if "nc" in _CACHE:
        return _CACHE["nc"]
    nc = bacc.Bacc("TRN2", target_bir_lowering=False, debug=False,
                   num_devices=N_CORES)
    n16 = sum(NCH16 * nt for (_, nt, _) in TGROUPS)
    n8 = sum(N_F8 * nt for (_, nt, _) in TGROUPS)
    xg16 = nc.dram_tensor("xg16", [P, n16], f16, kind="ExternalInput").ap()
    xg8 = nc.dram_tensor("xg8", [P, n8], f8, kind="ExternalInput").ap()
    wt = nc.dram_tensor("wt", [P, NCH * O], f16, kind="ExternalInput").ap()
    a = nc.dram_tensor("a", [R, D_IN], f16, kind="ExternalInput").ap()
    a2t = nc.dram_tensor("a2t", [P, NCH * R], f16, kind="ExternalInput").ap()
    g = nc.dram_tensor("g", [R, R], f16, kind="ExternalInput").ap()
    b2t = nc.dram_tensor("b2t", [R, O], f16, kind="ExternalInput").ap()
    mag = nc.dram_tensor("mag", [1, O], f32, kind="ExternalInput").ap()
    out = nc.dram_tensor("out", [TOKENS, O], f32, kind="ExternalOutput").ap()
    with tile.TileContext(nc) as tc:
        emit_kernel(nc, tc, xg16, xg8, wt, a, a2t, g, b2t, mag, out)
    nc.compile()
    _CACHE["nc"] = nc
    return nc


def prep_in_maps(x, lora_A_w, lora_B_w, base_w, magnitude):
    f8np = mybir.dt.np(f8)
    xf16 = x.astype(np.float16)
    x32 = x.astype(np.float32)
    # group-blocked x: fp16 leading chunks [p, (c t)], fp8 trailing
    # chunks pair-interleaved [p, (k j t)] — contiguous per-partition
    # slabs per token group for fat DMA descriptors
    g16_blocks = []
    g8_blocks = []
    for (t0, nt, _) in TGROUPS:
        b = xf16[t0:t0 + nt, :NCH16 * P].reshape(nt, NCH16, P)
        g16_blocks.append(b.transpose(2, 1, 0).reshape(P, NCH16 * nt))
        b8 = x32[t0:t0 + nt, NCH16 * P:].reshape(nt, NP8, 2, P)
        g8_blocks.append(
            np.clip(b8.transpose(3, 1, 2, 0), -240, 240)
            .astype(f8np).reshape(P, N_F8 * nt))
    xg16_np = np.ascontiguousarray(np.concatenate(g16_blocks, axis=1))
    xg8_np = np.ascontiguousarray(np.concatenate(g8_blocks, axis=1))
    a32 = lora_A_w.astype(np.float32)
    a_np = np.ascontiguousarray(a32.astype(np.float16))
    # (2A).T partition-major: a2t[p, c*R + r] = 2·A.T[c*128 + p, r]
    a2t_full = np.ascontiguousarray((2.0 * a32).astype(np.float16).T)
    a2t_np = np.ascontiguousarray(
        a2t_full.reshape(NCH, P, R).transpose(1, 0, 2).reshape(P, NCH * R))
    g_np = np.ascontiguousarray((a32 @ a32.T).astype(np.float16))
    in_maps = []
    for c in range(N_CORES):
        sl = slice(c * O, (c + 1) * O)
        # 16·W.T partition-major: wt_dev[p, c*O + o] = 16·W.T[c*128 + p, o]
        wt_sh = np.ascontiguousarray(
            (SC16 * base_w[sl].astype(np.float32)).astype(np.float16).T)
        wt_dev = np.ascontiguousarray(
            wt_sh.reshape(NCH, P, O).transpose(1, 0, 2).reshape(P, NCH * O))
        in_maps.append({
            "xg16": xg16_np,
            "xg8": xg8_np,
            "wt": wt_dev,
            "a": a_np,
            "a2t": a2t_np,
            "g": g_np,
            "b2t": np.ascontiguousarray(
                (SC16 * SCALING * lora_B_w[sl].astype(np.float32))
                .astype(np.float16).T),
            "mag": np.ascontiguousarray(
                (SC16 * magnitude[sl]).reshape(1, O).astype(np.float32)),
        })
    return in_maps


def kernel(x, lora_A_w, lora_B_w, base_w, magnitude):
    x = np.asarray(x)
    lora_A_w = np.asarray(lora_A_w)
    lora_B_w = np.asarray(lora_B_w)
    base_w = np.asarray(base_w)
    magnitude = np.asarray(magnitude)
    nc = build_nc()
    in_maps = prep_in_maps(x, lora_A_w, lora_B_w, base_w, magnitude)
    res = run_bass_kernel_spmd(nc, in_maps, list(range(N_CORES)))
    return np.concatenate(
        [res.results[c]["out"] for c in range(N_CORES)], axis=1)


# revision 12
# speedup vs baseline: 1.0099x; 1.0099x over previous
"""DoRA linear layer (nn_DoraLinearLayer) on 8 Trainium2 NeuronCores.

Math: out = (s-1)*(x @ W.T) + 2*s*((x @ A.T) @ B.T),
      s = magnitude / ||W + 2*B@A||_row  (stop-grad norm)

This factors exactly into ONE matmul per token: out = x @ Weff.T with
      n2     = ||W||²_row + Σ_r (2B).T ∘ (2A@W.T + G@(2B).T)   (G = A@A.T)
      s      = magnitude / sqrt(n2)
      Weff.T = (s-1) ∘ W.T + A.T @ (s∘(2B).T)
Device tensors carry a host-side ×16 scale (wt16, b2t16, mag16) so the
squares land in fp8e4m3 range; the 1/16 folds into the PSUM drain.
G is a 16×16 host-marshaled Gram of the rank-16 adapter.

Main loop is hybrid-precision: the leading 8 of 32 contraction chunks
are fp16 MMs; the trailing 24 run as 12 fp8e4m3 DoubleRow pair-MMs
(K=256 per instruction, ~1.13x the per-MM cost for 2x the contraction)
against weff tiles built directly in fp8. Validated numerically:
rel err ~1.8e-2 vs the 2e-2 gate (model matches HW to ~2%).

Queue discipline (each engine's DMA blocks its queue for the duration
of the in-flight transfer, so duties must not collide):
  sync:   wt chunks 0-15 (fat 4-chunk waves), then ALL fp16 x slabs
  gpsimd: wt chunks 16-31, then square/feeder-mul shares
  scalar: adapter smalls + ACT-table warmups, square share, s-chain
          sqrt/sm1, fp8 x slabs (split in half), drains + out DMAs
  vector: warmup memsets, square share, s-chain, weff feeder
The norm pipeline consumes chunks in the interleaved landing order
(sync and gpsimd waves alternate), a widened warmup burst keeps the
HAM clock at full rate through the DMA-paced phase, and filler
matmuls bridge the s-chain latency. The weff build trails 3 chunks
ahead of the chunk-major first token group; trailing-chunk weff goes
straight to fp8 (DVE tensor_add with fp8 output — no fp16 write, no
separate cast).

Sharding: column-parallel over out_features — core i owns rows
[i*512, (i+1)*512) of W/B/magnitude, x and A replicated, output shard
concatenated on the last dim on the host. Host-side work is marshaling
only: casts to fp16/fp8, transposes, slicing, static scaling, the
16×16 adapter Gram.
"""
import numpy as np

import concourse.bass as bass
import concourse.tile as tile
from concourse import bacc, mybir
from concourse.bass_utils import run_bass_kernel_spmd

N_CORES = 8
TOKENS, D_IN, D_OUT, R = 8192, 4096, 4096, 16
O = D_OUT // N_CORES          # 512 output features per core
P = 128                       # partitions
NCH = D_IN // P               # 32 contraction chunks
N_F8 = 24                     # trailing chunks computed in fp8 DoubleRow
NCH16 = NCH - N_F8            # leading fp16 chunks
NP8 = N_F8 // 2               # DoubleRow pair-MMs per tile
SCALING = 2.0                 # lora_alpha / r
SC16 = 16.0                   # static ×16 device scale (fp8 sq range)
N_WARM = 7                    # PE warmup matmuls (HAM ramp during DMA)

# token groups: first is chunk-major (7 PSUM banks) so the matmuls
# trail the weff feeder; the rest are tile-major; small last group
TGROUPS = [(0, 768, True)]
_t = 768
while _t + 512 <= TOKENS - 256:
    TGROUPS.append((_t, 512, False))
    _t += 512
TGROUPS.append((_t, 256, False))

f16 = mybir.dt.float16
f32 = mybir.dt.float32
f8 = mybir.dt.float8e4
Square = mybir.ActivationFunctionType.Square
Copy = mybir.ActivationFunctionType.Copy

_CACHE: dict = {}


def emit_kernel(nc, tc, xg16, xg8, wt, a, a2t, g, b2t, mag, out):
    """Emit the per-core program. All DRAM APs are per-core shapes."""
    from contextlib import ExitStack

    DoubleRow = mybir.MatmulPerfMode.DoubleRow

    with ExitStack() as ctx:
        singles = ctx.enter_context(tc.tile_pool(name="singles", bufs=1))
        setup = ctx.enter_context(tc.tile_pool(name="setup", bufs=8))
        # 8 PSUM banks: gen(6: warm + main mm) + lws(1) + nh(1: n2 rows
        # 0:16, h rows 32:48, then the full-width s broadcast)
        ps_gen = ctx.enter_context(tc.tile_pool(name="ps_gen", bufs=5, space="PSUM"))
        ps_lws = ctx.enter_context(tc.tile_pool(name="ps_lws", bufs=2, space="PSUM"))
        ps_nh = ctx.enter_context(tc.tile_pool(name="ps_nh", bufs=1, space="PSUM"))
        xpool = ctx.enter_context(tc.tile_pool(name="xpool", bufs=2))
        x0pool = ctx.enter_context(tc.tile_pool(name="x0pool", bufs=1))
        x8pool = ctx.enter_context(tc.tile_pool(name="x8pool", bufs=2))
        opool = ctx.enter_context(tc.tile_pool(name="opool", bufs=3))

        # ---- adapter smalls + ACT-table warmups on scalar: a2t first
        # (the h-chain needs it as soon as wt chunk 0 lands), then the
        # two table loads, then the rest of the smalls.
        a2t_sb = singles.tile([P, NCH, R], f16)
        nc.scalar.dma_start(out=a2t_sb, in_=a2t.rearrange("p (c r) -> p c r", r=R))
        act_warm = singles.tile([1, 1], f32)
        nc.vector.memset(act_warm, 1.0)
        act_warm2 = singles.tile([1, 1], f32)
        nc.scalar.sqrt(act_warm2, act_warm)
        act_warm3 = singles.tile([1, 1], f32)
        nc.scalar.activation(act_warm3, act_warm, Square)
        a_sb = singles.tile([R, D_IN], f16)
        nc.scalar.dma_start(out=a_sb, in_=a)
        b2t_sb = singles.tile([R, O], f16)
        nc.scalar.dma_start(out=b2t_sb, in_=b2t)
        g_sb = singles.tile([R, R], f16)
        nc.scalar.dma_start(out=g_sb, in_=g)
        mag_sb = singles.tile([1, O], f32)
        nc.scalar.dma_start(out=mag_sb, in_=mag)

        # ---- 16·W.T as fat 4-chunk waves: sync takes chunks 0-15,
        # gpsimd takes 16-31 (both queues stream concurrently; each
        # queue's later DMAs ring-block behind these, which is exactly
        # the priority we want).
        wt_sb = singles.tile([P, NCH, O], f16)
        wt_r = wt.rearrange("p (c o) -> p c o", o=O)
        wave_sizes = [2, 2, 4, 4, 4]
        lo = 0
        for wsz in wave_sizes:
            nc.sync.dma_start(out=wt_sb[:, lo:lo + wsz, :],
                              in_=wt_r[:, lo:lo + wsz, :])
            lo += wsz
        lo = 16
        for wsz in wave_sizes:
            nc.gpsimd.dma_start(out=wt_sb[:, lo:lo + wsz, :],
                                in_=wt_r[:, lo:lo + wsz, :])
            lo += wsz
        wt_t = [wt_sb[:, c, :] for c in range(NCH)]

        # ---- warmup operand memsets (vector queue) ----
        ones128 = singles.tile([P, P], f16)
        nc.vector.memset(ones128, 1.0)
        warm_rhs = singles.tile([P, O], f16)
        nc.vector.memset(warm_rhs, 0.002)
        ones8 = singles.tile([P, 2, 16], f8)
        nc.vector.memset(ones8, 1.0)
        ones16 = singles.tile([R, R], f16)
        nc.vector.memset(ones16, 1.0)
        ones_row32 = singles.tile([1, P], f32)
        nc.vector.memset(ones_row32, 1.0)

        # ---- PE warmup: full-K fp16 matmuls start the clock ramp and
        # keep HAM busy while the wt waves land
        warm_ps = ps_gen.tile([P, O], f32, name="gen")
        for _ in range(N_WARM):
            nc.tensor.matmul(warm_ps, lhsT=ones128, rhs=warm_rhs,
                             start=True, stop=True)

        # ---- norm pipeline, trailing the wt DMA in interleaved chunk
        # landing order (sync delivers 0..15, gpsimd 16..31):
        #   V/S/G: sq[c] = wt16[c]²  (fp8; shares 12/13/7 balance the
        #   engines' speeds and queue-free times)
        #   PE: h += (2A).T_c-major @ wt16[c]   (rows 32:48 of nh bank)
        #   PE: n2 += ones @ sq-pair  (fp8 DoubleRow, rows 0:16)
        sq_sb = singles.tile([P, NCH, O], f8)
        nh = ps_nh.tile([P, O], f32, name="nh")
        n2_ap = nh[0:16, :]
        h_ap = nh[32:48, :]

        c_seq = []
        for i in range(16):
            c_seq.append(i)
            c_seq.append(16 + i)
        # engine shares: vector 12, scalar 13, gpsimd 7 (by position)
        sq_eng = {}
        v_n = s_n = g_n = 0
        for p_, c in enumerate(c_seq):
            if g_n < 7 and p_ % 4 == 1:
                sq_eng[c] = "g"; g_n += 1
            elif s_n < 13 and p_ % 2 == 1:
                sq_eng[c] = "s"; s_n += 1
            elif v_n < 12:
                sq_eng[c] = "v"; v_n += 1
            else:
                sq_eng[c] = "s"; s_n += 1

        def emit_n2(k):
            nc.tensor.matmul(n2_ap, lhsT=ones8,
                             rhs=sq_sb[:, 2 * k:2 * k + 2, :],
                             perf_mode=DoubleRow,
                             start=(k == 0), stop=False)

        def fill_one():
            nc.tensor.matmul(warm_ps, lhsT=ones128, rhs=warm_rhs,
                             start=True, stop=True)

        pair_order = []
        for i in range(8):
            pair_order.append(i)
            pair_order.append(8 + i)
        sq_pos = {}
        n2_ptr = 0
        for p_, c in enumerate(c_seq):
            e = sq_eng[c]
            if e == "v":
                nc.vector.tensor_mul(sq_sb[:, c, :], wt_t[c], wt_t[c])
            elif e == "s":
                nc.scalar.activation(sq_sb[:, c, :], wt_t[c], Square)
            else:
                nc.gpsimd.tensor_mul(sq_sb[:, c, :], wt_t[c], wt_t[c])
            sq_pos[c] = p_
            nc.tensor.matmul(h_ap, lhsT=a2t_sb[:, c, :], rhs=wt_t[c],
                             start=(p_ == 0), stop=False)
            # emit n2 pairs as their squares age past a small lag
            if n2_ptr < len(pair_order):
                k = pair_order[n2_ptr]
                ca, cb = 2 * k, 2 * k + 1
                if (ca in sq_pos and cb in sq_pos
                        and p_ >= max(sq_pos[ca], sq_pos[cb]) + 6):
                    emit_n2(k)
                    n2_ptr += 1
            if p_ in (6, 10, 14, 18, 22, 24):
                fill_one()
        for k in pair_order[n2_ptr:]:
            emit_n2(k)

        # G-term folded onto the h chain, then the correction row-sum.
        # Filler matmuls between the dependency hops keep the PE dense
        # through the chain latency (an idle PE drops to half clock).
        def fill(n):
            for _ in range(n):
                nc.tensor.matmul(warm_ps, lhsT=ones128, rhs=warm_rhs,
                                 start=True, stop=True)

        nc.tensor.matmul(h_ap, lhsT=g_sb, rhs=b2t_sb, start=False, stop=True)
        fill(3)
        hterm = singles.tile([R, O], f16)
        nc.vector.tensor_mul(hterm, b2t_sb, h_ap)
        nc.tensor.matmul(n2_ap, lhsT=ones16, rhs=hterm, start=False, stop=True)
        fill(14)

        # ---- s = mag16 / sqrt(n2); broadcast into the nh bank ----
        nrm = singles.tile([1, O], f32)
        nc.scalar.sqrt(nrm, nh[0:1, :])
        rn = singles.tile([1, O], f32)
        nc.vector.reciprocal_approx_fast(out=rn, in_=nrm)
        s_row = singles.tile([1, O], f32)
        nc.vector.tensor_mul(s_row, mag_sb, rn)
        nc.tensor.matmul(nh, lhsT=ones_row32, rhs=s_row, start=True, stop=True)
        fill(6)
        b2st = singles.tile([R, O], f16)
        nc.vector.tensor_mul(b2st, b2t_sb, nh[0:R, :])
        sm1_sb = singles.tile([P, O], f32)
        nc.scalar.activation(sm1_sb, nh, Copy, bias=-1.0)

        # ---- x prefetch: all fp16 slabs on sync (ring-blocked behind
        # sync's wt waves = the priority we want); fp8 slabs on scalar,
        # split in half so a slab never blocks the drain stream long.
        off16 = []
        off8 = []
        o16 = o8 = 0
        for (t0_, nt_, _) in TGROUPS:
            off16.append(o16)
            off8.append(o8)
            o16 += NCH16 * nt_
            o8 += N_F8 * nt_

        def x16_ap(gi):
            nt_ = TGROUPS[gi][1]
            return xg16[:, off16[gi]: off16[gi] + NCH16 * nt_].rearrange(
                "p (c t) -> p c t", t=nt_)

        def x8_ap(gi):
            nt_ = TGROUPS[gi][1]
            return xg8[:, off8[gi]: off8[gi] + N_F8 * nt_].rearrange(
                "p (k j t) -> p k j t", j=2, t=nt_)

        def x8_fetch(gi, tile_):
            ap = x8_ap(gi)
            h1 = NP8 // 2
            nc.sync.dma_start(out=tile_[:, :h1], in_=ap[:, :h1])
            nc.sync.dma_start(out=tile_[:, h1:], in_=ap[:, h1:])

        ntok0 = TGROUPS[0][1]
        xt0 = x0pool.tile([P, NCH16, ntok0], f16, name="xt0")
        ap0 = x16_ap(0)
        for w in range(NCH16 // 4):
            lo, hi = 4 * w, 4 * w + 4
            nc.sync.dma_start(out=xt0[:, lo:hi, :], in_=ap0[:, lo:hi, :])
        x80 = x8pool.tile([P, NP8, 2, ntok0], f8, name="x8")
        x8_fetch(0, x80)
        ntok1 = TGROUPS[1][1]
        xt1 = xpool.tile([P, NCH16, ntok1], f16, name="xt")
        nc.sync.dma_start(out=xt1, in_=x16_ap(1))
        x81 = x8pool.tile([P, NP8, 2, ntok1], f8, name="x8")
        x8_fetch(1, x81)
        xt_pre = {0: (xt0, x80), 1: (xt1, x81)}

        # ---- weff build feeder: lws = A.T_c @ (s∘b2t16) on a rotating
        # bank. Leading (fp16) chunks: weff16[c] = sm1∘wt16[c] + lws in
        # place over wt16 (both DVE ops). Trailing chunks build straight
        # into fp8: tmp = sm1∘wt16[c] (vector/gpsimd alternating), then
        # weff8[c] = tmp + lws with fp8 output on the DVE.
        weff8 = singles.tile([P, N_F8, O], f8)

        def emit_weff(c):
            lws = ps_lws.tile([P, O], f32, name="lws")
            nc.tensor.matmul(lws, lhsT=a_sb[:, c * P:(c + 1) * P], rhs=b2st,
                             start=True, stop=True)
            tmp = setup.tile([P, O], f32, name="tmp")
            if c == 3 or c % 4 == 1:
                nc.vector.tensor_mul(tmp, wt_t[c], sm1_sb)
            else:
                nc.gpsimd.tensor_mul(tmp, wt_t[c], sm1_sb)
            if c < NCH16:
                nc.vector.tensor_add(wt_t[c], tmp, lws)
            else:
                nc.vector.tensor_add(weff8[:, c - NCH16, :], tmp, lws)

        weff_t = wt_t
        emit_weff(0)
        fill(4)
        emit_weff(1)
        fill(4)
        emit_weff(2)
        fill(5)

        def emit_pair_mm(ps, x8_t, m, k, stop):
            nc.tensor.matmul(
                ps,
                lhsT=x8_t[:, k, :, m * P: (m + 1) * P],
                rhs=weff8[:, 2 * k:2 * k + 2, :],
                perf_mode=DoubleRow,
                start=False, stop=stop,
            )

        for gi, (t0, ntok, chunk_major) in enumerate(TGROUPS):
            nm = ntok // P
            if gi in xt_pre:
                xt_t, x8_t = xt_pre[gi]
            else:
                xt_t = xpool.tile([P, NCH16, ntok], f16, name="xt")
                nc.sync.dma_start(out=xt_t, in_=x16_ap(gi))
                x8_t = x8pool.tile([P, NP8, 2, ntok], f8, name="x8")
                x8_fetch(gi, x8_t)
            if chunk_major:
                # the 6th accumulator reuses the norm bank (dead by now)
                pss = [ps_gen.tile([P, O], f32, name="gen") for _ in range(nm - 1)]
                pss.append(ps_nh.tile([P, O], f32, name="nh"))
                for c in range(NCH16):
                    if c + 3 < NCH:
                        emit_weff(c + 3)
                    for m in range(nm):
                        nc.tensor.matmul(
                            pss[m],
                            lhsT=xt_t[:, c, m * P: (m + 1) * P],
                            rhs=weff_t[c],
                            start=(c == 0), stop=False,
                        )
                # remaining weff builds interleave with the pair phase
                pend = list(range(NCH16 + 3, NCH))
                for fc in pend[:3]:
                    emit_weff(fc)
                pend = pend[3:]
                for k in range(NP8):
                    for fc in pend[:2]:
                        emit_weff(fc)
                    pend = pend[2:]
                    for m in range(nm):
                        emit_pair_mm(pss[m], x8_t, m, k, k == NP8 - 1)
                for m in range(nm):
                    ot = opool.tile([P, O], f32, name="ot")
                    nc.scalar.activation(ot, pss[m], Copy, scale=1.0 / SC16)
                    nc.scalar.dma_start(
                        out=out[t0 + m * P: t0 + (m + 1) * P, :], in_=ot)
            else:
                for m in range(nm):
                    ps = ps_gen.tile([P, O], f32, name="gen")
                    for c in range(NCH16):
                        nc.tensor.matmul(
                            ps,
                            lhsT=xt_t[:, c, m * P: (m + 1) * P],
                            rhs=weff_t[c],
                            start=(c == 0), stop=False,
                        )
                    for k in range(NP8):
                        emit_pair_mm(ps, x8_t, m, k, k == NP8 - 1)
                    ot = opool.tile([P, O], f32, name="ot")
                    last = (gi == len(TGROUPS) - 1 and m == nm - 1)
                    if last:
                        # final tile drains on the idle vector engine and
                        # sync queue so the tail isn't serialized behind
                        # the previous tile's scalar-queue work
                        nc.vector.tensor_scalar_mul(ot, ps, 1.0 / SC16)
                        nc.sync.dma_start(
                            out=out[t0 + m * P: t0 + (m + 1) * P, :], in_=ot)
                    else:
                        nc.scalar.activation(ot, ps, Copy, scale=1.0 / SC16)
                        nc.scalar.dma_start(
                            out=out[t0 + m * P: t0 + (m + 1) * P, :], in_=ot)


def build_nc():
    if "nc" in _CACHE:
        return _CACHE["nc"]
    nc = bacc.Bacc("TRN2", target_bir_lowering=False, debug=False,
                   num_devices=N_CORES)
    n16 = sum(NCH16 * nt for (_, nt, _) in TGROUPS)
    n8 = sum(N_F8 * nt for (_, nt, _) in TGROUPS)
    xg16 = nc.dram_tensor("xg16", [P, n16], f16, kind="ExternalInput").ap()
    xg8 = nc.dram_tensor("xg8", [P, n8], f8, kind="ExternalInput").ap()
    wt = nc.dram_tensor("wt", [P, NCH * O], f16, kind="ExternalInput").ap()
    a = nc.dram_tensor("a", [R, D_IN], f16, kind="ExternalInput").ap()
    a2t = nc.dram_tensor("a2t", [P, NCH * R], f16, kind="ExternalInput").ap()
    g = nc.dram_tensor("g", [R, R], f16, kind="ExternalInput").ap()
    b2t = nc.dram_tensor("b2t", [R, O], f16, kind="ExternalInput").ap()
    mag = nc.dram_tensor("mag", [1, O], f32, kind="ExternalInput").ap()
    out = nc.dram_tensor("out", [TOKENS, O], f32, kind="ExternalOutput").ap()
    with tile.TileContext(nc) as tc:
        emit_kernel(nc, tc, xg16, xg8, wt, a, a2t, g, b2t, mag, out)
    nc.compile()
    _CACHE["nc"] = nc
    return nc


def prep_in_maps(x, lora_A_w, lora_B_w, base_w, magnitude):
    f8np = mybir.dt.np(f8)
    xf16 = x.astype(np.float16)
    x32 = x.astype(np.float32)
    # group-blocked x: fp16 leading chunks [p, (c t)], fp8 trailing
    # chunks pair-interleaved [p, (k j t)] — contiguous per-partition
    # slabs per token group for fat DMA descriptors
    g16_blocks = []
    g8_blocks = []
    for (t0, nt, _) in TGROUPS:
        b = xf16[t0:t0 + nt, :NCH16 * P].reshape(nt, NCH16, P)
        g16_blocks.append(b.transpose(2, 1, 0).reshape(P, NCH16 * nt))
        b8 = x32[t0:t0 + nt, NCH16 * P:].reshape(nt, NP8, 2, P)
        g8_blocks.append(
            np.clip(b8.transpose(3, 1, 2, 0), -240, 240)
            .astype(f8np).reshape(P, N_F8 * nt))
    xg16_np = np.ascontiguousarray(np.concatenate(g16_blocks, axis=1))
    xg8_np = np.ascontiguousarray(np.concatenate(g8_blocks, axis=1))
    a32 = lora_A_w.astype(np.float32)
    a_np = np.ascontiguousarray(a32.astype(np.float16))
    # (2A).T partition-major: a2t[p, c*R + r] = 2·A.T[c*128 + p, r]
    a2t_full = np.ascontiguousarray((2.0 * a32).astype(np.float16).T)
    a2t_np = np.ascontiguousarray(
        a2t_full.reshape(NCH, P, R).transpose(1, 0, 2).reshape(P, NCH * R))
    g_np = np.ascontiguousarray((a32 @ a32.T).astype(np.float16))
    in_maps = []
    for c in range(N_CORES):
        sl = slice(c * O, (c + 1) * O)
        # 16·W.T partition-major: wt_dev[p, c*O + o] = 16·W.T[c*128 + p, o]
        wt_sh = np.ascontiguousarray(
            (SC16 * base_w[sl].astype(np.float32)).astype(np.float16).T)
        wt_dev = np.ascontiguousarray(
            wt_sh.reshape(NCH, P, O).transpose(1, 0, 2).reshape(P, NCH * O))
        in_maps.append({
            "xg16": xg16_np,
            "xg8": xg8_np,
            "wt": wt_dev,
            "a": a_np,
            "a2t": a2t_np,
            "g": g_np,
            "b2t": np.ascontiguousarray(
                (SC16 * SCALING * lora_B_w[sl].astype(np.float32))
                .astype(np.float16).T),
            "mag": np.ascontiguousarray(
                (SC16 * magnitude[sl]).reshape(1, O).astype(np.float32)),
        })
    return in_maps


def kernel(x, lora_A_w, lora_B_w, base_w, magnitude):
    x = np.asarray(x)
    lora_A_w = np.asarray(lora_A_w)
    lora_B_w = np.asarray(lora_B_w)
    base_w = np.asarray(base_w)
    magnitude = np.asarray(magnitude)
    nc = build_nc()
    in_maps = prep_in_maps(x, lora_A_w, lora_B_w, base_w, magnitude)
    res = run_bass_kernel_spmd(nc, in_maps, list(range(N_CORES)))
    return np.concatenate(
        [res.results[c]["out"] for c in range(N_CORES)], axis=1)


# revision 14
# speedup vs baseline: 1.0162x; 1.0062x over previous
"""DoRA linear layer (nn_DoraLinearLayer) on 8 Trainium2 NeuronCores.

Math: out = (s-1)*(x @ W.T) + 2*s*((x @ A.T) @ B.T),
      s = magnitude / ||W + 2*B@A||_row  (stop-grad norm)

This factors exactly into ONE matmul per token: out = x @ Weff.T with
      n2     = ||W||²_row + Σ_r (2B).T ∘ (2A@W.T + G@(2B).T)   (G = A@A.T)
      s      = magnitude / sqrt(n2)
      Weff.T = (s-1) ∘ W.T + A.T @ (s∘(2B).T)
Device tensors carry a host-side ×16 scale (wt16, b2t16, mag16) so the
squares land in fp8e4m3 range; the 1/16 folds into the PSUM drain.
G is a 16×16 host-marshaled Gram of the rank-16 adapter.

Main loop is hybrid-precision: the leading 8 of 32 contraction chunks
are fp16 MMs; the trailing 24 run as 12 fp8e4m3 DoubleRow pair-MMs
(K=256 per instruction, ~1.13x the per-MM cost for 2x the contraction)
against weff tiles built directly in fp8. Validated numerically:
rel err ~1.8e-2 vs the 2e-2 gate (model matches HW to ~2%).

Queue discipline (each engine's DMA blocks its queue for the duration
of the in-flight transfer, so duties must not collide):
  sync:   wt chunks 0-15 (fat 4-chunk waves), then ALL fp16 x slabs
  gpsimd: wt chunks 16-31, then square/feeder-mul shares
  scalar: adapter smalls + ACT-table warmups, square share, s-chain
          sqrt/sm1, fp8 x slabs (split in half), drains + out DMAs
  vector: warmup memsets, square share, s-chain, weff feeder
The norm pipeline consumes chunks in the interleaved landing order
(sync and gpsimd waves alternate), a widened warmup burst keeps the
HAM clock at full rate through the DMA-paced phase, and filler
matmuls bridge the s-chain latency. The weff build trails 3 chunks
ahead of the chunk-major first token group; trailing-chunk weff goes
straight to fp8 (DVE tensor_add with fp8 output — no fp16 write, no
separate cast).

Sharding: column-parallel over out_features — core i owns rows
[i*512, (i+1)*512) of W/B/magnitude, x and A replicated, output shard
concatenated on the last dim on the host. Host-side work is marshaling
only: casts to fp16/fp8, transposes, slicing, static scaling, the
16×16 adapter Gram.
"""
import numpy as np

import concourse.bass as bass
import concourse.tile as tile
from concourse import bacc, mybir
from concourse.bass_utils import run_bass_kernel_spmd

N_CORES = 8
TOKENS, D_IN, D_OUT, R = 8192, 4096, 4096, 16
O = D_OUT // N_CORES          # 512 output features per core
P = 128                       # partitions
NCH = D_IN // P               # 32 contraction chunks
N_F8 = 24                     # trailing chunks computed in fp8 DoubleRow
NCH16 = NCH - N_F8            # leading fp16 chunks
NP8 = N_F8 // 2               # DoubleRow pair-MMs per tile
SCALING = 2.0                 # lora_alpha / r
SC16 = 16.0                   # static ×16 device scale (fp8 sq range)
N_WARM = 7                    # PE warmup matmuls (HAM ramp during DMA)

# token groups: first is chunk-major (7 PSUM banks) so the matmuls
# trail the weff feeder; the rest are tile-major; small last group
TGROUPS = [(0, 768, True)]
_t = 768
while _t + 512 <= TOKENS - 256:
    TGROUPS.append((_t, 512, False))
    _t += 512
TGROUPS.append((_t, 256, False))

f16 = mybir.dt.float16
f32 = mybir.dt.float32
f8 = mybir.dt.float8e4
Square = mybir.ActivationFunctionType.Square
Copy = mybir.ActivationFunctionType.Copy

_CACHE: dict = {}


def emit_kernel(nc, tc, xg16, xg8, wt, a, a2t, g, b2t, mag, out):
    """Emit the per-core program. All DRAM APs are per-core shapes."""
    from contextlib import ExitStack

    DoubleRow = mybir.MatmulPerfMode.DoubleRow

    with ExitStack() as ctx:
        singles = ctx.enter_context(tc.tile_pool(name="singles", bufs=1))
        setup = ctx.enter_context(tc.tile_pool(name="setup", bufs=8))
        # 8 PSUM banks: gen(6: warm + main mm) + lws(1) + nh(1: n2 rows
        # 0:16, h rows 32:48, then the full-width s broadcast)
        ps_gen = ctx.enter_context(tc.tile_pool(name="ps_gen", bufs=5, space="PSUM"))
        ps_lws = ctx.enter_context(tc.tile_pool(name="ps_lws", bufs=2, space="PSUM"))
        ps_nh = ctx.enter_context(tc.tile_pool(name="ps_nh", bufs=1, space="PSUM"))
        xpool = ctx.enter_context(tc.tile_pool(name="xpool", bufs=2))
        x0pool = ctx.enter_context(tc.tile_pool(name="x0pool", bufs=1))
        x8pool = ctx.enter_context(tc.tile_pool(name="x8pool", bufs=2))
        opool = ctx.enter_context(tc.tile_pool(name="opool", bufs=3))

        # ---- adapter smalls + ACT-table warmups on scalar: a2t first
        # (the h-chain needs it as soon as wt chunk 0 lands), then the
        # two table loads, then the rest of the smalls.
        a2t_sb = singles.tile([P, NCH, R], f16)
        nc.scalar.dma_start(out=a2t_sb, in_=a2t.rearrange("p (c r) -> p c r", r=R))
        act_warm = singles.tile([1, 1], f32)
        nc.vector.memset(act_warm, 1.0)
        act_warm2 = singles.tile([1, 1], f32)
        nc.scalar.sqrt(act_warm2, act_warm)
        act_warm3 = singles.tile([1, 1], f32)
        nc.scalar.activation(act_warm3, act_warm, Square)
        a_sb = singles.tile([R, D_IN], f16)
        nc.scalar.dma_start(out=a_sb, in_=a)
        b2t_sb = singles.tile([R, O], f16)
        nc.scalar.dma_start(out=b2t_sb, in_=b2t)
        g_sb = singles.tile([R, R], f16)
        nc.scalar.dma_start(out=g_sb, in_=g)
        mag_sb = singles.tile([1, O], f32)
        nc.scalar.dma_start(out=mag_sb, in_=mag)

        # ---- 16·W.T as fat 4-chunk waves: sync takes chunks 0-15,
        # gpsimd takes 16-31 (both queues stream concurrently; each
        # queue's later DMAs ring-block behind these, which is exactly
        # the priority we want).
        ones128 = singles.tile([P, P], f16)
        nc.gpsimd.memset(ones128, 1.0)
        warm_rhs = singles.tile([P, O], f16)
        nc.gpsimd.memset(warm_rhs, 0.002)
        wt_sb = singles.tile([P, NCH, O], f16)
        wt_r = wt.rearrange("p (c o) -> p c o", o=O)
        wave_sizes = [2, 2, 4, 4, 4]
        lo = 0
        for wsz in wave_sizes:
            nc.sync.dma_start(out=wt_sb[:, lo:lo + wsz, :],
                              in_=wt_r[:, lo:lo + wsz, :])
            lo += wsz
        lo = 16
        for wsz in wave_sizes:
            nc.gpsimd.dma_start(out=wt_sb[:, lo:lo + wsz, :],
                                in_=wt_r[:, lo:lo + wsz, :])
            lo += wsz
        wt_t = [wt_sb[:, c, :] for c in range(NCH)]

        # ---- warmup operands (gpsimd memsets, emitted before its wt
        # waves so the PE warm burst starts right after the preamble)
        ones8 = singles.tile([P, 2, 16], f8)
        nc.vector.memset(ones8, 1.0)
        ones16 = singles.tile([R, R], f16)
        nc.vector.memset(ones16, 1.0)
        ones_row32 = singles.tile([1, P], f32)
        nc.vector.memset(ones_row32, 1.0)

        # ---- PE warmup: full-K fp16 matmuls start the clock ramp and
        # keep HAM busy while the wt waves land
        warm_ps = ps_gen.tile([P, O], f32, name="gen")
        for _ in range(N_WARM):
            nc.tensor.matmul(warm_ps, lhsT=ones128, rhs=warm_rhs,
                             start=True, stop=True)

        # ---- norm pipeline, trailing the wt DMA in interleaved chunk
        # landing order (sync delivers 0..15, gpsimd 16..31):
        #   V/S/G: sq[c] = wt16[c]²  (fp8; shares 12/13/7 balance the
        #   engines' speeds and queue-free times)
        #   PE: h += (2A).T_c-major @ wt16[c]   (rows 32:48 of nh bank)
        #   PE: n2 += ones @ sq-pair  (fp8 DoubleRow, rows 0:16)
        sq_sb = singles.tile([P, NCH, O], f8)
        nh = ps_nh.tile([P, O], f32, name="nh")
        n2_ap = nh[0:16, :]
        h_ap = nh[32:48, :]

        c_seq = []
        for i in range(16):
            c_seq.append(i)
            c_seq.append(16 + i)
        # engine shares: vector 12, scalar 13, gpsimd 7 (by position)
        sq_eng = {}
        v_n = s_n = g_n = 0
        for p_, c in enumerate(c_seq):
            if g_n < 7 and p_ % 4 == 1:
                sq_eng[c] = "g"; g_n += 1
            elif s_n < 13 and p_ % 2 == 1:
                sq_eng[c] = "s"; s_n += 1
            elif v_n < 12:
                sq_eng[c] = "v"; v_n += 1
            else:
                sq_eng[c] = "s"; s_n += 1

        def emit_n2(k):
            nc.tensor.matmul(n2_ap, lhsT=ones8,
                             rhs=sq_sb[:, 2 * k:2 * k + 2, :],
                             perf_mode=DoubleRow,
                             start=(k == 0), stop=False)

        def fill_one():
            nc.tensor.matmul(warm_ps, lhsT=ones128, rhs=warm_rhs,
                             start=True, stop=True)

        pair_order = []
        for i in range(8):
            pair_order.append(i)
            pair_order.append(8 + i)
        sq_pos = {}
        n2_ptr = 0
        for p_, c in enumerate(c_seq):
            e = sq_eng[c]
            if e == "v":
                nc.vector.tensor_mul(sq_sb[:, c, :], wt_t[c], wt_t[c])
            elif e == "s":
                nc.scalar.activation(sq_sb[:, c, :], wt_t[c], Square)
            else:
                nc.gpsimd.tensor_mul(sq_sb[:, c, :], wt_t[c], wt_t[c])
            sq_pos[c] = p_
            nc.tensor.matmul(h_ap, lhsT=a2t_sb[:, c, :], rhs=wt_t[c],
                             start=(p_ == 0), stop=False)
            # emit n2 pairs as their squares age past a small lag
            if n2_ptr < len(pair_order):
                k = pair_order[n2_ptr]
                ca, cb = 2 * k, 2 * k + 1
                if (ca in sq_pos and cb in sq_pos
                        and p_ >= max(sq_pos[ca], sq_pos[cb]) + 6):
                    emit_n2(k)
                    n2_ptr += 1
            if p_ in (6, 10, 14, 18, 22, 24):
                fill_one()
        for k in pair_order[n2_ptr:]:
            emit_n2(k)

        # G-term folded onto the h chain, then the correction row-sum.
        # Filler matmuls between the dependency hops keep the PE dense
        # through the chain latency (an idle PE drops to half clock).
        def fill(n):
            for _ in range(n):
                nc.tensor.matmul(warm_ps, lhsT=ones128, rhs=warm_rhs,
                                 start=True, stop=True)

        nc.tensor.matmul(h_ap, lhsT=g_sb, rhs=b2t_sb, start=False, stop=True)
        fill(3)
        hterm = singles.tile([R, O], f16)
        nc.vector.tensor_mul(hterm, b2t_sb, h_ap)
        nc.tensor.matmul(n2_ap, lhsT=ones16, rhs=hterm, start=False, stop=True)
        fill(14)

        # ---- s = mag16 / sqrt(n2); broadcast into the nh bank ----
        nrm = singles.tile([1, O], f32)
        nc.scalar.sqrt(nrm, nh[0:1, :])
        rn = singles.tile([1, O], f32)
        nc.vector.reciprocal_approx_fast(out=rn, in_=nrm)
        s_row = singles.tile([1, O], f32)
        nc.vector.tensor_mul(s_row, mag_sb, rn)
        nc.tensor.matmul(nh, lhsT=ones_row32, rhs=s_row, start=True, stop=True)
        fill(6)
        b2st = singles.tile([R, O], f16)
        nc.vector.tensor_mul(b2st, b2t_sb, nh[0:R, :])
        sm1_sb = singles.tile([P, O], f16)
        nc.scalar.activation(sm1_sb, nh, Copy, bias=-1.0)

        # ---- x prefetch: all fp16 slabs on sync (ring-blocked behind
        # sync's wt waves = the priority we want); fp8 slabs on scalar,
        # split in half so a slab never blocks the drain stream long.
        off16 = []
        off8 = []
        o16 = o8 = 0
        for (t0_, nt_, _) in TGROUPS:
            off16.append(o16)
            off8.append(o8)
            o16 += NCH16 * nt_
            o8 += N_F8 * nt_

        def x16_ap(gi):
            nt_ = TGROUPS[gi][1]
            return xg16[:, off16[gi]: off16[gi] + NCH16 * nt_].rearrange(
                "p (c t) -> p c t", t=nt_)

        def x8_ap(gi):
            nt_ = TGROUPS[gi][1]
            return xg8[:, off8[gi]: off8[gi] + N_F8 * nt_].rearrange(
                "p (k j t) -> p k j t", j=2, t=nt_)

        def x8_fetch(gi, tile_):
            ap = x8_ap(gi)
            h1 = NP8 // 2
            nc.sync.dma_start(out=tile_[:, :h1], in_=ap[:, :h1])
            nc.sync.dma_start(out=tile_[:, h1:], in_=ap[:, h1:])

        ntok0 = TGROUPS[0][1]
        xt0 = x0pool.tile([P, NCH16, ntok0], f16, name="xt0")
        ap0 = x16_ap(0)
        for w in range(NCH16 // 4):
            lo, hi = 4 * w, 4 * w + 4
            nc.sync.dma_start(out=xt0[:, lo:hi, :], in_=ap0[:, lo:hi, :])
        x80 = x8pool.tile([P, NP8, 2, ntok0], f8, name="x8")
        x8_fetch(0, x80)
        ntok1 = TGROUPS[1][1]
        xt1 = xpool.tile([P, NCH16, ntok1], f16, name="xt")
        nc.sync.dma_start(out=xt1, in_=x16_ap(1))
        x81 = x8pool.tile([P, NP8, 2, ntok1], f8, name="x8")
        x8_fetch(1, x81)
        xt_pre = {0: (xt0, x80), 1: (xt1, x81)}

        # ---- weff build feeder: lws = A.T_c @ (s∘b2t16) on a rotating
        # bank. Leading (fp16) chunks: weff16[c] = sm1∘wt16[c] + lws in
        # place over wt16 (both DVE ops). Trailing chunks build straight
        # into fp8: tmp = sm1∘wt16[c] (vector/gpsimd alternating), then
        # weff8[c] = tmp + lws with fp8 output on the DVE.
        weff8 = singles.tile([P, N_F8, O], f8)

        def emit_weff(c):
            lws = ps_lws.tile([P, O], f32, name="lws")
            nc.tensor.matmul(lws, lhsT=a_sb[:, c * P:(c + 1) * P], rhs=b2st,
                             start=True, stop=True)
            tmp = setup.tile([P, O], f16, name="tmp")
            if c == 3 or c % 4 == 1:
                nc.vector.tensor_mul(tmp, wt_t[c], sm1_sb)
            else:
                nc.gpsimd.tensor_mul(tmp, wt_t[c], sm1_sb)
            if c < NCH16:
                nc.vector.tensor_add(wt_t[c], tmp, lws)
            else:
                nc.vector.tensor_add(weff8[:, c - NCH16, :], tmp, lws)

        weff_t = wt_t
        emit_weff(0)
        fill(4)
        emit_weff(1)
        fill(4)
        emit_weff(2)
        fill(5)

        def emit_pair_mm(ps, x8_t, m, k, stop):
            nc.tensor.matmul(
                ps,
                lhsT=x8_t[:, k, :, m * P: (m + 1) * P],
                rhs=weff8[:, 2 * k:2 * k + 2, :],
                perf_mode=DoubleRow,
                start=False, stop=stop,
            )

        for gi, (t0, ntok, chunk_major) in enumerate(TGROUPS):
            nm = ntok // P
            if gi in xt_pre:
                xt_t, x8_t = xt_pre[gi]
            else:
                xt_t = xpool.tile([P, NCH16, ntok], f16, name="xt")
                nc.sync.dma_start(out=xt_t, in_=x16_ap(gi))
                x8_t = x8pool.tile([P, NP8, 2, ntok], f8, name="x8")
                x8_fetch(gi, x8_t)
            if chunk_major:
                # the 6th accumulator reuses the norm bank (dead by now)
                pss = [ps_gen.tile([P, O], f32, name="gen") for _ in range(nm - 1)]
                pss.append(ps_nh.tile([P, O], f32, name="nh"))
                for c in range(NCH16):
                    if c + 3 < NCH:
                        emit_weff(c + 3)
                    for m in range(nm):
                        nc.tensor.matmul(
                            pss[m],
                            lhsT=xt_t[:, c, m * P: (m + 1) * P],
                            rhs=weff_t[c],
                            start=(c == 0), stop=False,
                        )
                # remaining weff builds interleave with the pair phase
                pend = list(range(NCH16 + 3, NCH))
                for fc in pend[:3]:
                    emit_weff(fc)
                pend = pend[3:]
                for k in range(NP8):
                    for fc in pend[:2]:
                        emit_weff(fc)
                    pend = pend[2:]
                    for m in range(nm):
                        emit_pair_mm(pss[m], x8_t, m, k, k == NP8 - 1)
                for m in range(nm):
                    ot = opool.tile([P, O], f32, name="ot")
                    nc.scalar.activation(ot, pss[m], Copy, scale=1.0 / SC16)
                    nc.scalar.dma_start(
                        out=out[t0 + m * P: t0 + (m + 1) * P, :], in_=ot)
            else:
                for m in range(nm):
                    ps = ps_gen.tile([P, O], f32, name="gen")
                    for c in range(NCH16):
                        nc.tensor.matmul(
                            ps,
                            lhsT=xt_t[:, c, m * P: (m + 1) * P],
                            rhs=weff_t[c],
                            start=(c == 0), stop=False,
                        )
                    for k in range(NP8):
                        emit_pair_mm(ps, x8_t, m, k, k == NP8 - 1)
                    ot = opool.tile([P, O], f32, name="ot")
                    last = (gi == len(TGROUPS) - 1 and m == nm - 1)
                    if last:
                        # final tile drains on the idle vector engine and
                        # sync queue so the tail isn't serialized behind
                        # the previous tile's scalar-queue work
                        nc.vector.tensor_scalar_mul(ot, ps, 1.0 / SC16)
                        nc.sync.dma_start(
                            out=out[t0 + m * P: t0 + (m + 1) * P, :], in_=ot)
                    else:
                        nc.scalar.activation(ot, ps, Copy, scale=1.0 / SC16)
                        nc.scalar.dma_start(
                            out=out[t0 + m * P: t0 + (m + 1) * P, :], in_=ot)


def build_nc():
    if "nc" in _CACHE:
        return _CACHE["nc"]
    nc = bacc.Bacc("TRN2", target_bir_lowering=False, debug=False,
                   num_devices=N_CORES)
    n16 = sum(NCH16 * nt for (_, nt, _) in TGROUPS)
    n8 = sum(N_F8 * nt for (_, nt, _) in TGROUPS)
    xg16 = nc.dram_tensor("xg16", [P, n16], f16, kind="ExternalInput").ap()
    xg8 = nc.dram_tensor("xg8", [P, n8], f8, kind="ExternalInput").ap()
    wt = nc.dram_tensor("wt", [P, NCH * O], f16, kind="ExternalInput").ap()
    a = nc.dram_tensor("a", [R, D_IN], f16, kind="ExternalInput").ap()
    a2t = nc.dram_tensor("a2t", [P, NCH * R], f16, kind="ExternalInput").ap()
    g = nc.dram_tensor("g", [R, R], f16, kind="ExternalInput").ap()
    b2t = nc.dram_tensor("b2t", [R, O], f16, kind="ExternalInput").ap()
    mag = nc.dram_tensor("mag", [1, O], f32, kind="ExternalInput").ap()
    out = nc.dram_tensor("out", [TOKENS, O], f32, kind="ExternalOutput").ap()
    with tile.TileContext(nc) as tc:
        emit_kernel(nc, tc, xg16, xg8, wt, a, a2t, g, b2t, mag, out)
    nc.compile()
    _CACHE["nc"] = nc
    return nc


def prep_in_maps(x, lora_A_w, lora_B_w, base_w, magnitude):
    f8np = mybir.dt.np(f8)
    xf16 = x.astype(np.float16)
    x32 = x.astype(np.float32)
    # group-blocked x: fp16 leading chunks [p, (c t)], fp8 trailing
    # chunks pair-interleaved [p, (k j t)] — contiguous per-partition
    # slabs per token group for fat DMA descriptors
    g16_blocks = []
    g8_blocks = []
    for (t0, nt, _) in TGROUPS:
        b = xf16[t0:t0 + nt, :NCH16 * P].reshape(nt, NCH16, P)
        g16_blocks.append(b.transpose(2, 1, 0).reshape(P, NCH16 * nt))
        b8 = x32[t0:t0 + nt, NCH16 * P:].reshape(nt, NP8, 2, P)
        g8_blocks.append(
            np.clip(b8.transpose(3, 1, 2, 0), -240, 240)
            .astype(f8np).reshape(P, N_F8 * nt))
    xg16_np = np.ascontiguousarray(np.concatenate(g16_blocks, axis=1))
    xg8_np = np.ascontiguousarray(np.concatenate(g8_blocks, axis=1))
    a32 = lora_A_w.astype(np.float32)
    a_np = np.ascontiguousarray(a32.astype(np.float16))
    # (2A).T partition-major: a2t[p, c*R + r] = 2·A.T[c*128 + p, r]
    a2t_full = np.ascontiguousarray((2.0 * a32).astype(np.float16).T)
    a2t_np = np.ascontiguousarray(
        a2t_full.reshape(NCH, P, R).transpose(1, 0, 2).reshape(P, NCH * R))
    g_np = np.ascontiguousarray((a32 @ a32.T).astype(np.float16))
    in_maps = []
    for c in range(N_CORES):
        sl = slice(c * O, (c + 1) * O)
        # 16·W.T partition-major: wt_dev[p, c*O + o] = 16·W.T[c*128 + p, o]
        wt_sh = np.ascontiguousarray(
            (SC16 * base_w[sl].astype(np.float32)).astype(np.float16).T)
        wt_dev = np.ascontiguousarray(
            wt_sh.reshape(NCH, P, O).transpose(1, 0, 2).reshape(P, NCH * O))
        in_maps.append({
            "xg16": xg16_np,
            "xg8": xg8_np,
            "wt": wt_dev,
            "a": a_np,
            "a2t": a2t_np,
            "g": g_np,
            "b2t": np.ascontiguousarray(
                (SC16 * SCALING * lora_B_w[sl].astype(np.float32))
                .astype(np.float16).T),
            "mag": np.ascontiguousarray(
                (SC16 * magnitude[sl]).reshape(1, O).astype(np.float32)),
        })
    return in_maps


def kernel(x, lora_A_w, lora_B_w, base_w, magnitude):
    x = np.asarray(x)
    lora_A_w = np.asarray(lora_A_w)
    lora_B_w = np.asarray(lora_B_w)
    base_w = np.asarray(base_w)
    magnitude = np.asarray(magnitude)
    nc = build_nc()
    in_maps = prep_in_maps(x, lora_A_w, lora_B_w, base_w, magnitude)
    res = run_bass_kernel_spmd(nc, in_maps, list(range(N_CORES)))
    return np.concatenate(
        [res.results[c]["out"] for c in range(N_CORES)], axis=1)
